# revision 1
# baseline (speedup 1.0000x reference)
"""Trainium2 Bass kernel for nn_CorrectJAmbiguityBlock.

Mathematical structure (verified against the reference):
  - gather_idx / gather_idx2 enumerate, for every batch b and every edge
    (i<j), the 3x3 blocks (i,j) and (j,i) of the (3N,3N) matrix H.
  - The gathered blocks are conjugated by J = diag(1,1,s) with
    s = -1 if u_s[b, j] < 0 else +1, and scattered back to the SAME
    positions they were gathered from.  Off-diagonal blocks are covered
    exactly once; diagonal blocks are never written (stay zero).
  - (J H J)[r,c] = J[r,r] * H[r,c] * J[c,c], so the whole op is an
    elementwise mask:
       out[b, 3p+r, 3q+c] = H[b, 3p+r, 3q+c] * t[b, max(p,q)]^([r==2] ^ [c==2])
    for p != q, and 0 for p == q, where t = sign-ish (+1 / -1) of u_s.

Sharding: 8 cores = (batch b in 0..3) x (row half in 0..1).  Each core
processes a contiguous (768, 1536) slab of H; no cross-core traffic.

The device kernel multiplies H elementwise by the mask tensor.  The mask
is a deterministic function of the index structure and sign(u_s).
"""

import numpy as np

B, N = 4, 512
D = 3 * N           # 1536
HALF = D // 2       # 768 rows per core
NCORES = 8
PT = 128            # partitions per tile
NTILES = HALF // PT # 6

# ----------------------------------------------------------------------------
# Host-side index verification (insurance against unexpected index inputs)
# ----------------------------------------------------------------------------

_expected_idx_cache = {}


def _expected_indices():
    if "gi" not in _expected_idx_cache:
        i, j = np.triu_indices(N, k=1)
        M = i.shape[0]
        b = np.arange(B)[:, None, None, None]
        r = np.arange(3)[None, None, :, None]
        c = np.arange(3)[None, None, None, :]

        def pack(rows, cols):
            bb = np.broadcast_to(b, (B, M, 3, 3))
            rw = np.broadcast_to(rows, (B, M, 3, 3))
            cl = np.broadcast_to(cols, (B, M, 3, 3))
            return np.stack([bb, rw, cl], axis=-1).reshape(-1, 3).astype(np.int32)

        gi = pack(3 * i[None, :, None, None] + r, 3 * j[None, :, None, None] + c)
        gi2 = pack(3 * j[None, :, None, None] + r, 3 * i[None, :, None, None] + c)
        bu = np.broadcast_to(np.arange(B)[:, None], (B, M))
        ju = np.broadcast_to(j[None, :], (B, M))
        u_idx = np.stack([bu, ju], axis=-1).reshape(-1, 2).astype(np.int32)
        _expected_idx_cache["gi"] = gi
        _expected_idx_cache["gi2"] = gi2
        _expected_idx_cache["u_idx"] = u_idx
    return (
        _expected_idx_cache["gi"],
        _expected_idx_cache["gi2"],
        _expected_idx_cache["u_idx"],
    )


def _indices_match(gather_idx, gather_idx2, u_s_gather_idx):
    gi, gi2, u_idx = _expected_indices()
    return (
        gather_idx.shape == gi.shape
        and gather_idx2.shape == gi2.shape
        and u_s_gather_idx.shape == u_idx.shape
        and np.array_equal(gather_idx, gi)
        and np.array_equal(gather_idx2, gi2)
        and np.array_equal(u_s_gather_idx, u_idx)
    )


def _host_fallback(H, u_s, gather_idx, gather_idx2, u_s_gather_idx):
    """Generic (slow) host implementation for arbitrary indices."""
    H = np.asarray(H, dtype=np.float32)
    ind = (u_s[u_s_gather_idx[:, 0], u_s_gather_idx[:, 1]] < 0).reshape(B, -1)
    ind = ind.astype(H.dtype)
    Hg = H[gather_idx[:, 0], gather_idx[:, 1], gather_idx[:, 2]].reshape(B, -1, 3, 3)
    Hg2 = H[gather_idx2[:, 0], gather_idx2[:, 1], gather_idx2[:, 2]].reshape(B, -1, 3, 3)
    I3 = np.eye(3, dtype=H.dtype)
    Jd = np.diag(np.array([1.0, 1.0, -1.0], dtype=H.dtype))
    indb = ind[..., None, None]
    J = indb * Jd + (1.0 - indb) * I3
    Hc = np.einsum("bmij,bmjk,bmkl->bmil", J, Hg, J)
    Hc2 = np.einsum("bmij,bmjk,bmkl->bmil", J, Hg2, J)
    out = np.zeros_like(H)
    np.add.at(out, (gather_idx[:, 0], gather_idx[:, 1], gather_idx[:, 2]), Hc.reshape(-1))
    np.add.at(out, (gather_idx2[:, 0], gather_idx2[:, 1], gather_idx2[:, 2]), Hc2.reshape(-1))
    return out


# ----------------------------------------------------------------------------
# Host mask construction
# ----------------------------------------------------------------------------

_mask_static = {}


def _mask_statics():
    if "expo" not in _mask_static:
        e2 = (np.arange(D) % 3 == 2)
        blk = np.arange(D) // 3
        _mask_static["expo"] = e2[:, None] ^ e2[None, :]
        _mask_static["maxpq"] = np.maximum.outer(blk, blk).astype(np.int32)
        _mask_static["diag"] = blk[:, None] == blk[None, :]
    return _mask_static["expo"], _mask_static["maxpq"], _mask_static["diag"]


def _build_mask(u_s):
    """W (B, D, D) f32: the elementwise multiplier derived from sign(u_s)."""
    expo, maxpq, diag = _mask_statics()
    t = np.where(np.asarray(u_s) < 0, np.float32(-1.0), np.float32(1.0))
    W = np.empty((B, D, D), dtype=np.float32)
    for b in range(B):
        tb = t[b][maxpq]
        Wb = np.where(expo, tb, np.float32(1.0))
        Wb[diag] = 0.0
        W[b] = Wb
    return W


# ----------------------------------------------------------------------------
# Device program
# ----------------------------------------------------------------------------

_program_cache = {}


def _build_program(reps=1):
    """SPMD program for one core: out = h * w over a (768, 1536) slab."""
    import concourse.bacc as bacc
    import concourse.tile as tile
    import concourse.mybir as mybir

    f32 = mybir.dt.float32
    nc = bacc.Bacc("TRN2", target_bir_lowering=False, debug=False, num_devices=NCORES)
    h = nc.dram_tensor("h", [HALF, D], f32, kind="ExternalInput").ap()
    w = nc.dram_tensor("w", [HALF, D], f32, kind="ExternalInput").ap()
    o = nc.dram_tensor("o", [HALF, D], f32, kind="ExternalOutput").ap()

    with tile.TileContext(nc) as tc:
        with tc.tile_pool(name="hp", bufs=3) as hp, tc.tile_pool(name="wp", bufs=3) as wp:

            def body(_i=None):
                for k in range(NTILES):
                    rows = slice(k * PT, (k + 1) * PT)
                    th = hp.tile([PT, D], f32)
                    nc.sync.dma_start(th[:], h[rows, :])
                    tw = wp.tile([PT, D], f32)
                    nc.sync.dma_start(tw[:], w[rows, :])
                    nc.vector.tensor_mul(th[:], th[:], tw[:])
                    nc.sync.dma_start(o[rows, :], th[:])

            if reps == 1:
                body()
            else:
                with tc.For_i(0, reps, 1):
                    body()

    nc.compile()
    return nc


def _get_program(reps=1):
    if reps not in _program_cache:
        _program_cache[reps] = _build_program(reps)
    return _program_cache[reps]


def _run_on_device(H, W, reps=1, trace=False):
    from concourse.bass_utils import run_bass_kernel_spmd

    nc = _get_program(reps)
    in_maps = []
    for core in range(NCORES):
        b, half = divmod(core, 2)
        r0 = half * HALF
        in_maps.append(
            {
                "h": np.ascontiguousarray(H[b, r0 : r0 + HALF, :]),
                "w": np.ascontiguousarray(W[b, r0 : r0 + HALF, :]),
            }
        )
    res = run_bass_kernel_spmd(nc, in_maps, list(range(NCORES)), trace=trace)
    out = np.empty((B, D, D), dtype=np.float32)
    for core in range(NCORES):
        b, half = divmod(core, 2)
        r0 = half * HALF
        out[b, r0 : r0 + HALF, :] = res.results[core]["o"]
    return out, res


def kernel(H, u_s, gather_idx, gather_idx2, u_s_gather_idx):
    H = np.asarray(H, dtype=np.float32)
    u_s = np.asarray(u_s, dtype=np.float32)
    gather_idx = np.asarray(gather_idx)
    gather_idx2 = np.asarray(gather_idx2)
    u_s_gather_idx = np.asarray(u_s_gather_idx)

    if not _indices_match(gather_idx, gather_idx2, u_s_gather_idx):
        return _host_fallback(H, u_s, gather_idx, gather_idx2, u_s_gather_idx)

    W = _build_mask(u_s)
    out, _ = _run_on_device(H, W)
    return out


# revision 2
# speedup vs baseline: 111.5796x; 111.5796x over previous
"""Trainium2 Bass kernel for nn_CorrectJAmbiguityBlock.

Mathematical structure (verified against the reference):
  - gather_idx / gather_idx2 enumerate, for every batch b and every edge
    (i<j), the 3x3 blocks (i,j) and (j,i) of the (3N,3N) matrix H.
  - The gathered blocks are conjugated by J = diag(1,1,s) with
    s = -1 if u_s[b, j] < 0 else +1, and scattered back to the SAME
    positions they were gathered from.  Off-diagonal blocks are covered
    exactly once; diagonal blocks are never written (stay zero).
  - (J H J)[r,c] = J[r,r] * H[r,c] * J[c,c], so the whole op is an
    elementwise mask:
       out[b, 3p+r, 3q+c] = H[b, 3p+r, 3q+c] * t[b, max(p,q)]^([r==2] ^ [c==2])
    for p != q, and 0 for p == q, where t = where(u_s < 0, -1, 1).

Sharding: 8 cores = (batch b in 0..3) x (block parity hp in 0..1).  Core
(b, hp) owns the 3x3-block-rows with block index p = 2*i + hp of H[b]
(a (768, 1536) slab).  The parity interleave makes the per-tile
left/band/right column structure identical across cores, so a single
SPMD program serves all 8 cores; everything parity-dependent (sign
vectors, diagonal-band masks) is passed as per-core data.

Device algorithm per 128-block tile (rows grouped by row-class r=R%3):
  - load H rows (class r of blocks p0..p0+127) as a (128, 1536) tile
  - multiply the "flip candidate" positions (r==2 XOR c==2) by the edge
    sign: column-block sign tcb[q] right of the diagonal band, row-block
    sign tp[i] left of it, and a precomputed (Mlow*tp + Mup.tcb) weight
    inside the 256-block diagonal band (which also zeroes the diagonal
    3x3 blocks there)
  - zero remaining diagonal-block positions with a 0/1 mask
  - store the tile
All sign data is computed on device from u_s; the 0/1 masks are
constant structural data (like the index tensors themselves).
"""

import numpy as np

B, N = 4, 512
D = 3 * N            # 1536
HALF = D // 2        # 768 rows per core
NCORES = 8
PT = 128             # partitions per tile
NB = 256             # band width in blocks

# ----------------------------------------------------------------------------
# Host-side index verification (insurance against unexpected index inputs)
# ----------------------------------------------------------------------------

_expected_idx_cache = {}


def _expected_indices():
    if "gi" not in _expected_idx_cache:
        i, j = np.triu_indices(N, k=1)
        M = i.shape[0]
        b = np.arange(B)[:, None, None, None]
        r = np.arange(3)[None, None, :, None]
        c = np.arange(3)[None, None, None, :]

        def pack(rows, cols):
            bb = np.broadcast_to(b, (B, M, 3, 3))
            rw = np.broadcast_to(rows, (B, M, 3, 3))
            cl = np.broadcast_to(cols, (B, M, 3, 3))
            return np.stack([bb, rw, cl], axis=-1).reshape(-1, 3).astype(np.int32)

        gi = pack(3 * i[None, :, None, None] + r, 3 * j[None, :, None, None] + c)
        gi2 = pack(3 * j[None, :, None, None] + r, 3 * i[None, :, None, None] + c)
        bu = np.broadcast_to(np.arange(B)[:, None], (B, M))
        ju = np.broadcast_to(j[None, :], (B, M))
        u_idx = np.stack([bu, ju], axis=-1).reshape(-1, 2).astype(np.int32)
        _expected_idx_cache["gi"] = gi
        _expected_idx_cache["gi2"] = gi2
        _expected_idx_cache["u_idx"] = u_idx
    return (
        _expected_idx_cache["gi"],
        _expected_idx_cache["gi2"],
        _expected_idx_cache["u_idx"],
    )


def _indices_match(gather_idx, gather_idx2, u_s_gather_idx):
    gi, gi2, u_idx = _expected_indices()
    return (
        gather_idx.shape == gi.shape
        and gather_idx2.shape == gi2.shape
        and u_s_gather_idx.shape == u_idx.shape
        and np.array_equal(gather_idx, gi)
        and np.array_equal(gather_idx2, gi2)
        and np.array_equal(u_s_gather_idx, u_idx)
    )


def _host_fallback(H, u_s, gather_idx, gather_idx2, u_s_gather_idx):
    """Generic (slow) host implementation for arbitrary indices."""
    H = np.asarray(H, dtype=np.float32)
    ind = (u_s[u_s_gather_idx[:, 0], u_s_gather_idx[:, 1]] < 0).reshape(B, -1)
    ind = ind.astype(H.dtype)
    Hg = H[gather_idx[:, 0], gather_idx[:, 1], gather_idx[:, 2]].reshape(B, -1, 3, 3)
    Hg2 = H[gather_idx2[:, 0], gather_idx2[:, 1], gather_idx2[:, 2]].reshape(B, -1, 3, 3)
    I3 = np.eye(3, dtype=H.dtype)
    Jd = np.diag(np.array([1.0, 1.0, -1.0], dtype=H.dtype))
    indb = ind[..., None, None]
    J = indb * Jd + (1.0 - indb) * I3
    Hc = np.einsum("bmij,bmjk,bmkl->bmil", J, Hg, J)
    Hc2 = np.einsum("bmij,bmjk,bmkl->bmil", J, Hg2, J)
    out = np.zeros_like(H)
    np.add.at(out, (gather_idx[:, 0], gather_idx[:, 1], gather_idx[:, 2]), Hc.reshape(-1))
    np.add.at(out, (gather_idx2[:, 0], gather_idx2[:, 1], gather_idx2[:, 2]), Hc2.reshape(-1))
    return out


# ----------------------------------------------------------------------------
# Per-core constant masks (structural, parity-dependent)
# ----------------------------------------------------------------------------

_mask_cache = {}


def _core_masks(hp):
    """(128, 1280) f32: [Mlow | Mup | Mnodiag | Mnodiag2] for parity hp."""
    if hp not in _mask_cache:
        i = np.arange(PT)[:, None]
        dq = np.arange(NB)[None, :]
        diagpos = dq == 2 * i + hp
        mlow = (dq < 2 * i + hp).astype(np.float32)
        mup = (dq > 2 * i + hp).astype(np.float32)
        mnodiag = 1.0 - diagpos.astype(np.float32)
        mnodiag2 = np.repeat(mnodiag, 2, axis=1)
        _mask_cache[hp] = np.ascontiguousarray(
            np.concatenate([mlow, mup, mnodiag, mnodiag2], axis=1)
        )
    return _mask_cache[hp]


# ----------------------------------------------------------------------------
# Device program (SPMD, identical for all 8 cores)
# ----------------------------------------------------------------------------

_program_cache = {}


def _build_program(reps=1):
    import concourse.bacc as bacc
    import concourse.tile as tile
    import concourse.mybir as mybir
    from concourse.alu_op_type import AluOpType

    f32 = mybir.dt.float32
    MULT, ADD, IS_LT = AluOpType.mult, AluOpType.add, AluOpType.is_lt

    nc = bacc.Bacc("TRN2", target_bir_lowering=False, debug=False, num_devices=NCORES)
    h = nc.dram_tensor("h", [HALF, D], f32, kind="ExternalInput").ap()
    us = nc.dram_tensor("us", [N], f32, kind="ExternalInput").ap()
    uss = nc.dram_tensor("uss", [NB], f32, kind="ExternalInput").ap()
    mk = nc.dram_tensor("mk", [PT, 5 * NB], f32, kind="ExternalInput").ap()
    o = nc.dram_tensor("o", [HALF, D], f32, kind="ExternalOutput").ap()

    hr = h.rearrange("(i r) c -> r i c", r=3)
    orr = o.rearrange("(i r) c -> r i c", r=3)

    with tile.TileContext(nc) as tc:
        with (
            tc.tile_pool(name="const", bufs=1) as const,
            tc.tile_pool(name="data", bufs=4) as pool,
        ):
            # --- one-time setup -------------------------------------------
            mkT = const.tile([PT, 5 * NB], f32)
            nc.sync.dma_start(mkT[:], mk[:])
            mlow = mkT[:, 0:NB]
            mup = mkT[:, NB : 2 * NB]
            mnodiag = mkT[:, 2 * NB : 3 * NB]
            mnodiag2 = mkT[:, 3 * NB : 5 * NB].rearrange("p (q c) -> p q c", c=2)

            # tcb[i, q] = sign of u_s[b, q], replicated across partitions
            tcb = const.tile([PT, N], f32)
            nc.sync.dma_start(tcb[:], us[None, :].partition_broadcast(PT))
            nc.vector.tensor_scalar(tcb[:], tcb[:], 0.0, None, IS_LT)
            nc.vector.tensor_scalar(tcb[:], tcb[:], -2.0, 1.0, MULT, ADD)

            # tpt[i, t] = sign of u_s[b, 2*(128*t + i) + hp]  (own-block sign)
            tpt = const.tile([PT, 2], f32)
            nc.sync.dma_start(tpt[:], uss.rearrange("(t i) -> i t", i=PT))
            nc.vector.tensor_scalar(tpt[:], tpt[:], 0.0, None, IS_LT)
            nc.vector.tensor_scalar(tpt[:], tpt[:], -2.0, 1.0, MULT, ADD)

            # wb[:, 256t:256t+256]: band weight = Mlow*tp_t + Mup*tcb_band
            # (zero at own-diagonal position)
            wb = const.tile([PT, 2 * NB], f32)
            for t in (0, 1):
                wbt = wb[:, NB * t : NB * (t + 1)]
                nc.vector.tensor_mul(wbt, mup, tcb[:, NB * t : NB * (t + 1)])
                nc.vector.scalar_tensor_tensor(
                    wbt, mlow, tpt[:, t : t + 1], wbt, MULT, ADD
                )

            # --- main body ------------------------------------------------
            def body(_i=None):
                for t in (0, 1):
                    c0 = HALF * t          # band start column
                    wbt = wb[:, NB * t : NB * (t + 1)]
                    for k in (0, 1, 2):
                        th = pool.tile([PT, D], f32)
                        nc.sync.dma_start(th[:], hr[k, PT * t : PT * (t + 1), :])
                        if k < 2:
                            # flip candidates: c % 3 == 2
                            if t == 0:
                                sl = th[:, HALF + 2 : D : 3]
                                nc.vector.tensor_mul(sl, sl, tcb[:, NB:N])
                            else:
                                sl = th[:, 2:HALF:3]
                                nc.vector.tensor_scalar(sl, sl, tpt[:, 1:2], None, MULT)
                            bnd = th[:, c0 + 2 : c0 + HALF : 3]
                            nc.vector.tensor_mul(bnd, bnd, wbt)
                            d01 = th[:, c0 : c0 + HALF].rearrange(
                                "p (q c) -> p q c", c=3
                            )[:, :, 0:2]
                            nc.vector.tensor_mul(d01, d01, mnodiag2)
                        else:
                            # flip candidates: c % 3 in {0, 1}
                            if t == 0:
                                for cc in (0, 1):
                                    sl = th[:, HALF + cc : D : 3]
                                    nc.vector.tensor_mul(sl, sl, tcb[:, NB:N])
                            else:
                                sl = th[:, 0:HALF].rearrange("p (q c) -> p q c", c=3)[
                                    :, :, 0:2
                                ]
                                nc.vector.tensor_scalar(sl, sl, tpt[:, 1:2], None, MULT)
                            for cc in (0, 1):
                                bnd = th[:, c0 + cc : c0 + HALF : 3]
                                nc.vector.tensor_mul(bnd, bnd, wbt)
                            b2 = th[:, c0 + 2 : c0 + HALF : 3]
                            nc.vector.tensor_mul(b2, b2, mnodiag)
                        nc.sync.dma_start(orr[k, PT * t : PT * (t + 1), :], th[:])

            if reps == 1:
                body()
            else:
                with tc.For_i(0, reps, 1):
                    body()

    nc.compile()
    return nc


def _get_program(reps=1):
    if reps not in _program_cache:
        _program_cache[reps] = _build_program(reps)
    return _program_cache[reps]


# ----------------------------------------------------------------------------
# Host orchestration
# ----------------------------------------------------------------------------


def _core_inputs(H, u_s, core):
    b, hp = divmod(core, 2)
    hb = H[b].reshape(NB, 6, D)[:, 3 * hp : 3 * hp + 3, :].reshape(HALF, D)
    return {
        "h": np.ascontiguousarray(hb),
        "us": np.ascontiguousarray(u_s[b]),
        "uss": np.ascontiguousarray(u_s[b][hp::2]),
        "mk": _core_masks(hp),
    }


def _assemble(results):
    out = np.empty((B, D, D), dtype=np.float32)
    for core in range(NCORES):
        b, hp = divmod(core, 2)
        out[b].reshape(NB, 6, D)[:, 3 * hp : 3 * hp + 3, :] = results[core][
            "o"
        ].reshape(NB, 3, D)
    return out


def _run_on_device(H, u_s, reps=1, trace=False):
    from concourse.bass_utils import run_bass_kernel_spmd

    nc = _get_program(reps)
    in_maps = [_core_inputs(H, u_s, core) for core in range(NCORES)]
    res = run_bass_kernel_spmd(nc, in_maps, list(range(NCORES)), trace=trace)
    return _assemble(res.results), res


def kernel(H, u_s, gather_idx, gather_idx2, u_s_gather_idx):
    H = np.asarray(H, dtype=np.float32)
    u_s = np.asarray(u_s, dtype=np.float32)
    gather_idx = np.asarray(gather_idx)
    gather_idx2 = np.asarray(gather_idx2)
    u_s_gather_idx = np.asarray(u_s_gather_idx)

    if not _indices_match(gather_idx, gather_idx2, u_s_gather_idx):
        return _host_fallback(H, u_s, gather_idx, gather_idx2, u_s_gather_idx)

    out, _ = _run_on_device(H, u_s)
    return out


# revision 3
# speedup vs baseline: 113.0960x; 1.0136x over previous
"""Trainium2 Bass kernel for nn_CorrectJAmbiguityBlock.

Mathematical structure (verified against the reference):
  - gather_idx / gather_idx2 enumerate, for every batch b and every edge
    (i<j), the 3x3 blocks (i,j) and (j,i) of the (3N,3N) matrix H.
  - The gathered blocks are conjugated by J = diag(1,1,s) with
    s = -1 if u_s[b, j] < 0 else +1, and scattered back to the SAME
    positions they were gathered from.  Off-diagonal blocks are covered
    exactly once; diagonal blocks are never written (stay zero).
  - (J H J)[r,c] = J[r,r] * H[r,c] * J[c,c], so the whole op is an
    elementwise mask:
       out[b, 3p+r, 3q+c] = H[b, 3p+r, 3q+c] * t[b, max(p,q)]^([r==2] ^ [c==2])
    for p != q, and 0 for p == q, where t = where(u_s < 0, -1, 1).

Sharding: 8 cores = (batch b in 0..3) x (block parity hp in 0..1).  Core
(b, hp) owns the 3x3-block-rows with block index p = 2*i + hp of H[b]
(a (768, 1536) slab).  The parity interleave makes the per-tile
left/band/right column structure identical across cores, so a single
SPMD program serves all 8 cores; everything parity-dependent (sign
vectors, diagonal-band masks) is passed as per-core data.

Device algorithm per 128-block tile (rows grouped by row-class r=R%3):
  - load H rows (class r of blocks p0..p0+127) as a (128, 1536) tile
  - multiply the "flip candidate" positions (r==2 XOR c==2) by the edge
    sign: column-block sign tcb[q] right of the diagonal band, row-block
    sign tp[i] left of it, and a precomputed (Mlow*tp + Mup.tcb) weight
    inside the 256-block diagonal band (which also zeroes the diagonal
    3x3 blocks there)
  - zero remaining diagonal-block positions with a 0/1 mask
  - store the tile
All sign data is computed on device from u_s; the 0/1 masks are
constant structural data (like the index tensors themselves).
"""

import numpy as np

B, N = 4, 512
D = 3 * N            # 1536
HALF = D // 2        # 768 rows per core
NCORES = 8
PT = 128             # partitions per tile
NB = 256             # band width in blocks

# ----------------------------------------------------------------------------
# Host-side index verification (insurance against unexpected index inputs)
# ----------------------------------------------------------------------------

_expected_idx_cache = {}


def _expected_indices():
    if "gi" not in _expected_idx_cache:
        i, j = np.triu_indices(N, k=1)
        M = i.shape[0]
        b = np.arange(B)[:, None, None, None]
        r = np.arange(3)[None, None, :, None]
        c = np.arange(3)[None, None, None, :]

        def pack(rows, cols):
            bb = np.broadcast_to(b, (B, M, 3, 3))
            rw = np.broadcast_to(rows, (B, M, 3, 3))
            cl = np.broadcast_to(cols, (B, M, 3, 3))
            return np.stack([bb, rw, cl], axis=-1).reshape(-1, 3).astype(np.int32)

        gi = pack(3 * i[None, :, None, None] + r, 3 * j[None, :, None, None] + c)
        gi2 = pack(3 * j[None, :, None, None] + r, 3 * i[None, :, None, None] + c)
        bu = np.broadcast_to(np.arange(B)[:, None], (B, M))
        ju = np.broadcast_to(j[None, :], (B, M))
        u_idx = np.stack([bu, ju], axis=-1).reshape(-1, 2).astype(np.int32)
        _expected_idx_cache["gi"] = gi
        _expected_idx_cache["gi2"] = gi2
        _expected_idx_cache["u_idx"] = u_idx
    return (
        _expected_idx_cache["gi"],
        _expected_idx_cache["gi2"],
        _expected_idx_cache["u_idx"],
    )


def _indices_match(gather_idx, gather_idx2, u_s_gather_idx):
    gi, gi2, u_idx = _expected_indices()
    return (
        gather_idx.shape == gi.shape
        and gather_idx2.shape == gi2.shape
        and u_s_gather_idx.shape == u_idx.shape
        and np.array_equal(gather_idx, gi)
        and np.array_equal(gather_idx2, gi2)
        and np.array_equal(u_s_gather_idx, u_idx)
    )


def _host_fallback(H, u_s, gather_idx, gather_idx2, u_s_gather_idx):
    """Generic (slow) host implementation for arbitrary indices."""
    H = np.asarray(H, dtype=np.float32)
    ind = (u_s[u_s_gather_idx[:, 0], u_s_gather_idx[:, 1]] < 0).reshape(B, -1)
    ind = ind.astype(H.dtype)
    Hg = H[gather_idx[:, 0], gather_idx[:, 1], gather_idx[:, 2]].reshape(B, -1, 3, 3)
    Hg2 = H[gather_idx2[:, 0], gather_idx2[:, 1], gather_idx2[:, 2]].reshape(B, -1, 3, 3)
    I3 = np.eye(3, dtype=H.dtype)
    Jd = np.diag(np.array([1.0, 1.0, -1.0], dtype=H.dtype))
    indb = ind[..., None, None]
    J = indb * Jd + (1.0 - indb) * I3
    Hc = np.einsum("bmij,bmjk,bmkl->bmil", J, Hg, J)
    Hc2 = np.einsum("bmij,bmjk,bmkl->bmil", J, Hg2, J)
    out = np.zeros_like(H)
    np.add.at(out, (gather_idx[:, 0], gather_idx[:, 1], gather_idx[:, 2]), Hc.reshape(-1))
    np.add.at(out, (gather_idx2[:, 0], gather_idx2[:, 1], gather_idx2[:, 2]), Hc2.reshape(-1))
    return out


# ----------------------------------------------------------------------------
# Per-core constant masks (structural, parity-dependent)
# ----------------------------------------------------------------------------

_mask_cache = {}


def _core_masks(hp):
    """(128, 1280) f32: [Mlow | Mup | Mnodiag | Mnodiag2] for parity hp."""
    if hp not in _mask_cache:
        i = np.arange(PT)[:, None]
        dq = np.arange(NB)[None, :]
        diagpos = dq == 2 * i + hp
        mlow = (dq < 2 * i + hp).astype(np.float32)
        mup = (dq > 2 * i + hp).astype(np.float32)
        mnodiag = 1.0 - diagpos.astype(np.float32)
        mnodiag2 = np.repeat(mnodiag, 2, axis=1)
        _mask_cache[hp] = np.ascontiguousarray(
            np.concatenate([mlow, mup, mnodiag, mnodiag2], axis=1)
        )
    return _mask_cache[hp]


# ----------------------------------------------------------------------------
# Device program (SPMD, identical for all 8 cores)
# ----------------------------------------------------------------------------

_program_cache = {}


def _build_program(reps=1):
    import concourse.bacc as bacc
    import concourse.tile as tile
    import concourse.mybir as mybir
    from concourse.alu_op_type import AluOpType

    f32 = mybir.dt.float32
    MULT, ADD, IS_LT = AluOpType.mult, AluOpType.add, AluOpType.is_lt

    nc = bacc.Bacc("TRN2", target_bir_lowering=False, debug=False, num_devices=NCORES)
    h = nc.dram_tensor("h", [HALF, D], f32, kind="ExternalInput").ap()
    us = nc.dram_tensor("us", [N], f32, kind="ExternalInput").ap()
    uss = nc.dram_tensor("uss", [NB], f32, kind="ExternalInput").ap()
    mk = nc.dram_tensor("mk", [PT, 5 * NB], f32, kind="ExternalInput").ap()
    o = nc.dram_tensor("o", [HALF, D], f32, kind="ExternalOutput").ap()

    # block-row view: (block-group i, class r, col c); per-partition rows are
    # 3 consecutive slab rows = 18 KiB contiguous, so each tile is ONE
    # 2.25 MiB DMA (>=1 MiB for peak DMA efficiency)
    hb = h.rearrange("(i r) c -> i r c", r=3)
    ob = o.rearrange("(i r) c -> i r c", r=3)

    with tile.TileContext(nc) as tc:
        with (
            tc.tile_pool(name="const", bufs=1) as const,
            tc.tile_pool(name="data", bufs=4) as pool,
        ):
            # --- one-time setup -------------------------------------------
            mkT = const.tile([PT, 5 * NB], f32)
            nc.sync.dma_start(mkT[:], mk[:])
            mlow = mkT[:, 0:NB]
            mup = mkT[:, NB : 2 * NB]
            mnodiag = mkT[:, 2 * NB : 3 * NB]
            mnodiag2 = mkT[:, 3 * NB : 5 * NB]

            # tcb[i, q] = sign of u_s[b, q], replicated across partitions
            tcb = const.tile([PT, N], f32)
            nc.sync.dma_start(tcb[:], us[None, :].partition_broadcast(PT))
            nc.vector.tensor_scalar(tcb[:], tcb[:], 0.0, None, IS_LT)
            nc.vector.tensor_scalar(tcb[:], tcb[:], -2.0, 1.0, MULT, ADD)

            # tpt[i, t] = sign of u_s[b, 2*(128*t + i) + hp]  (own-block sign)
            tpt = const.tile([PT, 2], f32)
            nc.sync.dma_start(tpt[:], uss.rearrange("(t i) -> i t", i=PT))
            nc.vector.tensor_scalar(tpt[:], tpt[:], 0.0, None, IS_LT)
            nc.vector.tensor_scalar(tpt[:], tpt[:], -2.0, 1.0, MULT, ADD)

            # wb_t = Mlow*tp_t + Mup*tcb_band (zero at own-diagonal block)
            wb = const.tile([PT, 2 * NB], f32)
            # fused full-band weights, one 768-wide multiply per class group:
            #   w3a_t[3dq+cc] = mnodiag[dq] for cc<2, wb_t[dq] for cc==2   (r<2 rows)
            #   w3b_t[3dq+cc] = wb_t[dq]   for cc<2, mnodiag[dq] for cc==2 (r==2 rows)
            w3a = const.tile([PT, 2 * HALF], f32)
            w3b = const.tile([PT, 2 * HALF], f32)
            for t in (0, 1):
                wbt = wb[:, NB * t : NB * (t + 1)]
                nc.vector.tensor_mul(wbt, mup, tcb[:, NB * t : NB * (t + 1)])
                nc.vector.scalar_tensor_tensor(
                    wbt, mlow, tpt[:, t : t + 1], wbt, MULT, ADD
                )
                w3at = w3a[:, HALF * t : HALF * (t + 1)].rearrange(
                    "p (q c) -> p q c", c=3
                )
                w3bt = w3b[:, HALF * t : HALF * (t + 1)].rearrange(
                    "p (q c) -> p q c", c=3
                )
                m2v = mnodiag2.rearrange("p (q c) -> p q c", c=2)
                wb2v = wbt.rearrange("p (q c) -> p q c", c=1).broadcast_to([PT, NB, 2])
                nc.vector.tensor_copy(w3at[:, :, 0:2], m2v)
                nc.vector.tensor_copy(w3at[:, :, 2:3], wbt.rearrange("p (q c) -> p q c", c=1))
                nc.vector.tensor_copy(w3bt[:, :, 0:2], wb2v)
                nc.vector.tensor_copy(w3bt[:, :, 2:3], mnodiag.rearrange("p (q c) -> p q c", c=1))

            tcb2r = (
                tcb[:, NB:N]
                .rearrange("p (q c) -> p q c", c=1)
                .broadcast_to([PT, NB, 2])
            )

            # --- main body ------------------------------------------------
            def body(_i=None):
                for t in (0, 1):
                    c0 = HALF * t          # band start column within a class row
                    tb = pool.tile([PT, 3 * D], f32)
                    nc.sync.dma_start(
                        tb[:].rearrange("p (r c) -> p r c", c=D),
                        hb[PT * t : PT * (t + 1)],
                    )
                    for k in (0, 1):
                        K = D * k
                        if t == 0:
                            sl = tb[:, K + HALF + 2 : K + D : 3]
                            nc.vector.tensor_mul(sl, sl, tcb[:, NB:N])
                        else:
                            sl = tb[:, K + 2 : K + HALF : 3]
                            nc.vector.tensor_scalar(sl, sl, tpt[:, 1:2], None, MULT)
                        bnd = tb[:, K + c0 : K + c0 + HALF]
                        nc.vector.tensor_mul(
                            bnd, bnd, w3a[:, HALF * t : HALF * (t + 1)]
                        )
                    K = 2 * D
                    if t == 0:
                        sl = tb[:, K + HALF : K + D].rearrange(
                            "p (q c) -> p q c", c=3
                        )[:, :, 0:2]
                        nc.vector.tensor_tensor(sl, sl, tcb2r, MULT)
                    else:
                        sl = tb[:, K : K + HALF].rearrange("p (q c) -> p q c", c=3)[
                            :, :, 0:2
                        ]
                        nc.vector.tensor_scalar(sl, sl, tpt[:, 1:2], None, MULT)
                    bnd = tb[:, K + c0 : K + c0 + HALF]
                    nc.vector.tensor_mul(bnd, bnd, w3b[:, HALF * t : HALF * (t + 1)])
                    nc.sync.dma_start(
                        ob[PT * t : PT * (t + 1)],
                        tb[:].rearrange("p (r c) -> p r c", c=D),
                    )

            if reps == 1:
                body()
            else:
                with tc.For_i(0, reps, 1):
                    body()

    nc.compile()
    return nc


def _get_program(reps=1):
    if reps not in _program_cache:
        _program_cache[reps] = _build_program(reps)
    return _program_cache[reps]


# ----------------------------------------------------------------------------
# Host orchestration
# ----------------------------------------------------------------------------


def _core_inputs(H, u_s, core):
    b, hp = divmod(core, 2)
    hb = H[b].reshape(NB, 6, D)[:, 3 * hp : 3 * hp + 3, :].reshape(HALF, D)
    return {
        "h": np.ascontiguousarray(hb),
        "us": np.ascontiguousarray(u_s[b]),
        "uss": np.ascontiguousarray(u_s[b][hp::2]),
        "mk": _core_masks(hp),
    }


def _assemble(results):
    out = np.empty((B, D, D), dtype=np.float32)
    for core in range(NCORES):
        b, hp = divmod(core, 2)
        out[b].reshape(NB, 6, D)[:, 3 * hp : 3 * hp + 3, :] = results[core][
            "o"
        ].reshape(NB, 3, D)
    return out


def _run_on_device(H, u_s, reps=1, trace=False):
    from concourse.bass_utils import run_bass_kernel_spmd

    nc = _get_program(reps)
    in_maps = [_core_inputs(H, u_s, core) for core in range(NCORES)]
    res = run_bass_kernel_spmd(nc, in_maps, list(range(NCORES)), trace=trace)
    return _assemble(res.results), res


def kernel(H, u_s, gather_idx, gather_idx2, u_s_gather_idx):
    H = np.asarray(H, dtype=np.float32)
    u_s = np.asarray(u_s, dtype=np.float32)
    gather_idx = np.asarray(gather_idx)
    gather_idx2 = np.asarray(gather_idx2)
    u_s_gather_idx = np.asarray(u_s_gather_idx)

    if not _indices_match(gather_idx, gather_idx2, u_s_gather_idx):
        return _host_fallback(H, u_s, gather_idx, gather_idx2, u_s_gather_idx)

    out, _ = _run_on_device(H, u_s)
    return out


# revision 4
# speedup vs baseline: 114.5125x; 1.0125x over previous
"""Trainium2 Bass kernel for nn_CorrectJAmbiguityBlock.

Mathematical structure (verified against the reference):
  - gather_idx / gather_idx2 enumerate, for every batch b and every edge
    (i<j), the 3x3 blocks (i,j) and (j,i) of the (3N,3N) matrix H.
  - The gathered blocks are conjugated by J = diag(1,1,s) with
    s = -1 if u_s[b, j] < 0 else +1, and scattered back to the SAME
    positions they were gathered from.  Off-diagonal blocks are covered
    exactly once; diagonal blocks are never written (stay zero).
  - (J H J)[r,c] = J[r,r] * H[r,c] * J[c,c], so the whole op is an
    elementwise mask:
       out[b, 3p+r, 3q+c] = H[b, 3p+r, 3q+c] * t[b, max(p,q)]^([r==2] ^ [c==2])
    for p != q, and 0 for p == q, where t = where(u_s < 0, -1, 1).

Sharding: 8 cores = (batch b in 0..3) x (block parity hp in 0..1).  Core
(b, hp) owns the 3x3-block-rows with block index p = 2*i + hp of H[b]
(a (768, 1536) slab).  The parity interleave makes the per-tile
left/band/right column structure identical across cores, so a single
SPMD program serves all 8 cores; everything parity-dependent (sign
vectors, diagonal-band masks) is passed as per-core data.

Device algorithm per 128-block tile (rows grouped by row-class r=R%3):
  - load H rows (class r of blocks p0..p0+127) as a (128, 1536) tile
  - multiply the "flip candidate" positions (r==2 XOR c==2) by the edge
    sign: column-block sign tcb[q] right of the diagonal band, row-block
    sign tp[i] left of it, and a precomputed (Mlow*tp + Mup.tcb) weight
    inside the 256-block diagonal band (which also zeroes the diagonal
    3x3 blocks there)
  - zero remaining diagonal-block positions with a 0/1 mask
  - store the tile
All sign data is computed on device from u_s; the 0/1 masks are
constant structural data (like the index tensors themselves).
"""

import numpy as np

B, N = 4, 512
D = 3 * N            # 1536
HALF = D // 2        # 768 rows per core
NCORES = 8
PT = 128             # partitions per tile
NB = 256             # band width in blocks

# ----------------------------------------------------------------------------
# Host-side index verification (insurance against unexpected index inputs)
# ----------------------------------------------------------------------------

_expected_idx_cache = {}


def _expected_indices():
    if "gi" not in _expected_idx_cache:
        i, j = np.triu_indices(N, k=1)
        M = i.shape[0]
        b = np.arange(B)[:, None, None, None]
        r = np.arange(3)[None, None, :, None]
        c = np.arange(3)[None, None, None, :]

        def pack(rows, cols):
            bb = np.broadcast_to(b, (B, M, 3, 3))
            rw = np.broadcast_to(rows, (B, M, 3, 3))
            cl = np.broadcast_to(cols, (B, M, 3, 3))
            return np.stack([bb, rw, cl], axis=-1).reshape(-1, 3).astype(np.int32)

        gi = pack(3 * i[None, :, None, None] + r, 3 * j[None, :, None, None] + c)
        gi2 = pack(3 * j[None, :, None, None] + r, 3 * i[None, :, None, None] + c)
        bu = np.broadcast_to(np.arange(B)[:, None], (B, M))
        ju = np.broadcast_to(j[None, :], (B, M))
        u_idx = np.stack([bu, ju], axis=-1).reshape(-1, 2).astype(np.int32)
        _expected_idx_cache["gi"] = gi
        _expected_idx_cache["gi2"] = gi2
        _expected_idx_cache["u_idx"] = u_idx
    return (
        _expected_idx_cache["gi"],
        _expected_idx_cache["gi2"],
        _expected_idx_cache["u_idx"],
    )


def _indices_match(gather_idx, gather_idx2, u_s_gather_idx):
    gi, gi2, u_idx = _expected_indices()
    return (
        gather_idx.shape == gi.shape
        and gather_idx2.shape == gi2.shape
        and u_s_gather_idx.shape == u_idx.shape
        and np.array_equal(gather_idx, gi)
        and np.array_equal(gather_idx2, gi2)
        and np.array_equal(u_s_gather_idx, u_idx)
    )


def _host_fallback(H, u_s, gather_idx, gather_idx2, u_s_gather_idx):
    """Generic (slow) host implementation for arbitrary indices."""
    H = np.asarray(H, dtype=np.float32)
    ind = (u_s[u_s_gather_idx[:, 0], u_s_gather_idx[:, 1]] < 0).reshape(B, -1)
    ind = ind.astype(H.dtype)
    Hg = H[gather_idx[:, 0], gather_idx[:, 1], gather_idx[:, 2]].reshape(B, -1, 3, 3)
    Hg2 = H[gather_idx2[:, 0], gather_idx2[:, 1], gather_idx2[:, 2]].reshape(B, -1, 3, 3)
    I3 = np.eye(3, dtype=H.dtype)
    Jd = np.diag(np.array([1.0, 1.0, -1.0], dtype=H.dtype))
    indb = ind[..., None, None]
    J = indb * Jd + (1.0 - indb) * I3
    Hc = np.einsum("bmij,bmjk,bmkl->bmil", J, Hg, J)
    Hc2 = np.einsum("bmij,bmjk,bmkl->bmil", J, Hg2, J)
    out = np.zeros_like(H)
    np.add.at(out, (gather_idx[:, 0], gather_idx[:, 1], gather_idx[:, 2]), Hc.reshape(-1))
    np.add.at(out, (gather_idx2[:, 0], gather_idx2[:, 1], gather_idx2[:, 2]), Hc2.reshape(-1))
    return out


# ----------------------------------------------------------------------------
# Per-core constant masks (structural, parity-dependent)
# ----------------------------------------------------------------------------

_mask_cache = {}


def _core_masks(hp):
    """(128, 1280) f32: [Mlow | Mup | Mnodiag | Mnodiag2] for parity hp."""
    if hp not in _mask_cache:
        i = np.arange(PT)[:, None]
        dq = np.arange(NB)[None, :]
        diagpos = dq == 2 * i + hp
        mlow = (dq < 2 * i + hp).astype(np.float32)
        mup = (dq > 2 * i + hp).astype(np.float32)
        mnodiag = 1.0 - diagpos.astype(np.float32)
        mnodiag2 = np.repeat(mnodiag, 2, axis=1)
        _mask_cache[hp] = np.ascontiguousarray(
            np.concatenate([mlow, mup, mnodiag, mnodiag2], axis=1)
        )
    return _mask_cache[hp]


# ----------------------------------------------------------------------------
# Device program (SPMD, identical for all 8 cores)
# ----------------------------------------------------------------------------

_program_cache = {}


def _build_program(reps=1):
    import concourse.bacc as bacc
    import concourse.tile as tile
    import concourse.mybir as mybir
    from concourse.alu_op_type import AluOpType

    f32 = mybir.dt.float32
    MULT, ADD, IS_LT = AluOpType.mult, AluOpType.add, AluOpType.is_lt

    nc = bacc.Bacc("TRN2", target_bir_lowering=False, debug=False, num_devices=NCORES)
    h = nc.dram_tensor("h", [HALF, D], f32, kind="ExternalInput").ap()
    us = nc.dram_tensor("us", [N], f32, kind="ExternalInput").ap()
    uss = nc.dram_tensor("uss", [NB], f32, kind="ExternalInput").ap()
    mk = nc.dram_tensor("mk", [PT, 5 * NB], f32, kind="ExternalInput").ap()
    o = nc.dram_tensor("o", [HALF, D], f32, kind="ExternalOutput").ap()

    # block-row view: (block-group i, class r, col c); per-partition rows are
    # 3 consecutive slab rows = 18 KiB contiguous, so each tile is ONE
    # 2.25 MiB DMA (>=1 MiB for peak DMA efficiency)
    hb = h.rearrange("(i r) c -> i r c", r=3)
    ob = o.rearrange("(i r) c -> i r c", r=3)

    with tile.TileContext(nc) as tc:
        with (
            tc.tile_pool(name="const", bufs=1) as const,
            tc.tile_pool(name="data", bufs=4) as pool,
        ):
            # --- one-time setup -------------------------------------------
            mkT = const.tile([PT, 5 * NB], f32)
            nc.sync.dma_start(mkT[:], mk[:])
            mlow = mkT[:, 0:NB]
            mup = mkT[:, NB : 2 * NB]
            mnodiag = mkT[:, 2 * NB : 3 * NB]
            mnodiag2 = mkT[:, 3 * NB : 5 * NB]

            # tcb[i, q] = sign of u_s[b, q], replicated across partitions
            tcb = const.tile([PT, N], f32)
            nc.sync.dma_start(tcb[:], us[None, :].partition_broadcast(PT))
            nc.vector.tensor_scalar(tcb[:], tcb[:], 0.0, None, IS_LT)
            nc.vector.tensor_scalar(tcb[:], tcb[:], -2.0, 1.0, MULT, ADD)

            # tpt[i, t] = sign of u_s[b, 2*(128*t + i) + hp]  (own-block sign)
            tpt = const.tile([PT, 2], f32)
            nc.sync.dma_start(tpt[:], uss.rearrange("(t i) -> i t", i=PT))
            nc.vector.tensor_scalar(tpt[:], tpt[:], 0.0, None, IS_LT)
            nc.vector.tensor_scalar(tpt[:], tpt[:], -2.0, 1.0, MULT, ADD)

            # wb_t = Mlow*tp_t + Mup*tcb_band (zero at own-diagonal block)
            wb = const.tile([PT, 2 * NB], f32)
            # fused full-band weights, one 768-wide multiply per class group:
            #   w3a_t[3dq+cc] = mnodiag[dq] for cc<2, wb_t[dq] for cc==2   (r<2 rows)
            #   w3b_t[3dq+cc] = wb_t[dq]   for cc<2, mnodiag[dq] for cc==2 (r==2 rows)
            w3a = const.tile([PT, 2 * HALF], f32)
            w3b = const.tile([PT, 2 * HALF], f32)
            for t in (0, 1):
                wbt = wb[:, NB * t : NB * (t + 1)]
                nc.vector.tensor_mul(wbt, mup, tcb[:, NB * t : NB * (t + 1)])
                nc.vector.scalar_tensor_tensor(
                    wbt, mlow, tpt[:, t : t + 1], wbt, MULT, ADD
                )
                w3at = w3a[:, HALF * t : HALF * (t + 1)].rearrange(
                    "p (q c) -> p q c", c=3
                )
                w3bt = w3b[:, HALF * t : HALF * (t + 1)].rearrange(
                    "p (q c) -> p q c", c=3
                )
                m2v = mnodiag2.rearrange("p (q c) -> p q c", c=2)
                wb2v = wbt.rearrange("p (q c) -> p q c", c=1).broadcast_to([PT, NB, 2])
                nc.vector.tensor_copy(w3at[:, :, 0:2], m2v)
                nc.vector.tensor_copy(w3at[:, :, 2:3], wbt.rearrange("p (q c) -> p q c", c=1))
                nc.vector.tensor_copy(w3bt[:, :, 0:2], wb2v)
                nc.vector.tensor_copy(w3bt[:, :, 2:3], mnodiag.rearrange("p (q c) -> p q c", c=1))

            tcb2r = (
                tcb[:, NB:N]
                .rearrange("p (q c) -> p q c", c=1)
                .broadcast_to([PT, NB, 2])
            )

            # --- main body ------------------------------------------------
            def body(_i=None):
                for t in (0, 1):
                    c0 = HALF * t          # band start column within a class row
                    tb = pool.tile([PT, 3 * D], f32)
                    nc.sync.dma_start(
                        tb[:].rearrange("p (r c) -> p r c", c=D),
                        hb[PT * t : PT * (t + 1)],
                    )
                    for k in (0, 1):
                        K = D * k
                        if t == 0:
                            sl = tb[:, K + HALF + 2 : K + D : 3]
                            nc.vector.tensor_mul(sl, sl, tcb[:, NB:N])
                        else:
                            sl = tb[:, K + 2 : K + HALF : 3]
                            nc.vector.tensor_scalar(sl, sl, tpt[:, 1:2], None, MULT)
                        bnd = tb[:, K + c0 : K + c0 + HALF]
                        nc.vector.tensor_mul(
                            bnd, bnd, w3a[:, HALF * t : HALF * (t + 1)]
                        )
                    K = 2 * D
                    if t == 0:
                        sl = tb[:, K + HALF : K + D].rearrange(
                            "p (q c) -> p q c", c=3
                        )[:, :, 0:2]
                        nc.vector.tensor_tensor(sl, sl, tcb2r, MULT)
                    else:
                        sl = tb[:, K : K + HALF].rearrange("p (q c) -> p q c", c=3)[
                            :, :, 0:2
                        ]
                        nc.vector.tensor_scalar(sl, sl, tpt[:, 1:2], None, MULT)
                    bnd = tb[:, K + c0 : K + c0 + HALF]
                    nc.vector.tensor_mul(bnd, bnd, w3b[:, HALF * t : HALF * (t + 1)])
                    # stores on the ACT HW-DGE ring, loads on the SP ring:
                    # keeps both physical descriptor-generation rings busy
                    nc.scalar.dma_start(
                        ob[PT * t : PT * (t + 1)],
                        tb[:].rearrange("p (r c) -> p r c", c=D),
                    )

            if reps == 1:
                body()
            else:
                with tc.For_i(0, reps, 1):
                    body()

    nc.compile()
    return nc


def _get_program(reps=1):
    if reps not in _program_cache:
        _program_cache[reps] = _build_program(reps)
    return _program_cache[reps]


# ----------------------------------------------------------------------------
# Host orchestration
# ----------------------------------------------------------------------------


def _core_inputs(H, u_s, core):
    b, hp = divmod(core, 2)
    hb = H[b].reshape(NB, 6, D)[:, 3 * hp : 3 * hp + 3, :].reshape(HALF, D)
    return {
        "h": np.ascontiguousarray(hb),
        "us": np.ascontiguousarray(u_s[b]),
        "uss": np.ascontiguousarray(u_s[b][hp::2]),
        "mk": _core_masks(hp),
    }


def _assemble(results):
    out = np.empty((B, D, D), dtype=np.float32)
    for core in range(NCORES):
        b, hp = divmod(core, 2)
        out[b].reshape(NB, 6, D)[:, 3 * hp : 3 * hp + 3, :] = results[core][
            "o"
        ].reshape(NB, 3, D)
    return out


def _run_on_device(H, u_s, reps=1, trace=False):
    from concourse.bass_utils import run_bass_kernel_spmd

    nc = _get_program(reps)
    in_maps = [_core_inputs(H, u_s, core) for core in range(NCORES)]
    res = run_bass_kernel_spmd(nc, in_maps, list(range(NCORES)), trace=trace)
    return _assemble(res.results), res


def kernel(H, u_s, gather_idx, gather_idx2, u_s_gather_idx):
    H = np.asarray(H, dtype=np.float32)
    u_s = np.asarray(u_s, dtype=np.float32)
    gather_idx = np.asarray(gather_idx)
    gather_idx2 = np.asarray(gather_idx2)
    u_s_gather_idx = np.asarray(u_s_gather_idx)

    if not _indices_match(gather_idx, gather_idx2, u_s_gather_idx):
        return _host_fallback(H, u_s, gather_idx, gather_idx2, u_s_gather_idx)

    out, _ = _run_on_device(H, u_s)
    return out
